# revision 62
# baseline (speedup 1.0000x reference)
"""Multi-head attention (B=4, N=2048, E=1024, H=16, D=64) on 8 TRN2 NeuronCores.

Sharding: core c = (batch b = c//2, head-half hh = c%2). Each core computes,
for its batch, 8 heads worth of Q/K/V projections (a 512-column slice of
Wq/Wk/Wv), full-sequence attention for those heads, and the partial output
projection through the matching 512-row slice of Wo. The host sums the two
partial outputs per batch and adds the closed-form bias correction
(bv/512) @ Wo + bo (each softmax row sums to exactly 1/512 after the
reference's divide-by-E/2).

Host-side prep: x arrives already transposed ([E, N]) and cast to f16, the
weight slices arrive f16, and the Q/K biases arrive as [128, OCH] f32
columns -- so the kernel has no transposes, no input casts, and applies the
bias on the DVE during the PSUM->SBUF copy (tensor_scalar_add) instead of
rank-1 PE matmuls. x^T streams in over two DMA queues (sync/gpsimd) in
token quarters, with the V projection and K chunk 0 interleaved per quarter
so the PE starts within a few us and stays dense.

Layout: Q^T/K^T live [e_out, tok] so the scores are computed transposed
(S^T = K Q^T) with the softmax denominator folded in as a 512-valued
column of V_aug (so the PSUM Z row is already scaled by E/2). exp runs on
ScalarE straight out of PSUM (no max subtraction -- scores are ~N(0,8),
fp32 exp never overflows). Head pairs run concurrently on PE row halves
0-63/64-127 (tile positions via base partitions), sharing one [128, 1024]
S^T PSUM tile so a single exp covers both heads. The exp stream paces the
steady state (~1.1us per key tile); all projection work (K/Q for later
pairs, output projection of finished quarters) is chopped into
single-matmul "filler" closures popped a few per key-tile step, so the PE
never bursts long enough to starve ScalarE.

Per-PAIR normalization: quick copies park both heads' pre-scaled Z rows
in zbuf (partitions 0/64) and both O^T halves in one [128, 512] tile; a
single K=128 indicator matmul (ind128) broadcasts both Z rows across
their 64-partition halves (K=128 so its LDWEIGHTS hides under the
surrounding K=128 AV/filler streams -- the PE serializes weight loads
whenever the row-group class changes, so the old rank-1 K=1 broadcasts
cost two exposed LDW transitions each), reciprocal_approx_fast inverts
straight out of PSUM, and one DVE multiply lands both heads of oT.
Drains are deferred into the next pair's loop.

Tail: quarter 3's outproj is pre-accumulated over head chunks 0-2 and
streamed to out_d during the last pair (fill=5 drains those filler fns
in-pair); the last chunk's products go to a separate outc3 output that
the host folds in (it already sums partial outputs across cores). The
last pair's PSUM-freeing copies split across ScalarE/DVE (ScalarE is
done with exps by then) and throwaway matmuls hold the PE clock-gate
(HAM) open through the drain so the tail matmuls run at 2.4 GHz.
A 6-matmul warmup on a zeroed tile likewise opens the clock gate at
~7us, before the first projection's DMA dependencies land.

Reference quirk handled here: scores are NOT scaled by 1/sqrt(d); the
softmax output is divided by E/2 = 512.
"""

import collections
import sys

if "/opt/trn_rl_repo" not in sys.path:
    sys.path.insert(0, "/opt/trn_rl_repo")

import numpy as np

B, N, E, H = 4, 2048, 1024, 16
D = E // H          # 64
P = 128             # partitions
EH = E // 2         # 512: per-core e_out slice
HL = 8              # heads per core
ECH = E // P        # 8 e_in chunks
OCH = EH // P       # 4 e_out chunks
KC = N // P         # 16 key/token tiles
QH = 4              # q quarters per head pass
QHW = N // QH       # 512
MV = 512            # moving free dim (PSUM bank limit: 512 fp32)
FILL = 2            # filler closures popped per key-tile step
DRAIN_GPSIMD = False  # GpSimd partition_broadcast crashes the exec unit; use PE

_CACHE = {}


def _build():
    import concourse.bass as bass  # noqa: F401  (side-effect imports)
    import concourse.tile as tile
    from concourse import bacc, mybir

    f32 = mybir.dt.float32
    f16 = mybir.dt.float16
    bf16 = mybir.dt.bfloat16
    Exp = mybir.ActivationFunctionType.Exp
    mult = mybir.AluOpType.mult
    add = mybir.AluOpType.add

    nc = bacc.Bacc("TRN2", target_bir_lowering=False, debug=False)

    xt_d = nc.dram_tensor("xt", [E, N], f16, kind="ExternalInput").ap()
    wq_d = nc.dram_tensor("wq", [E, EH], f16, kind="ExternalInput").ap()
    wk_d = nc.dram_tensor("wk", [E, EH], f16, kind="ExternalInput").ap()
    wv_d = nc.dram_tensor("wv", [E, EH], f16, kind="ExternalInput").ap()
    wo_d = nc.dram_tensor("wo", [EH, E], f16, kind="ExternalInput").ap()
    bq_d = nc.dram_tensor("bqc", [P, OCH], f32, kind="ExternalInput").ap()
    bk_d = nc.dram_tensor("bkc", [P, OCH], f32, kind="ExternalInput").ap()
    # fp16 outputs: each core's contribution is a partial sum the host
    # accumulates in fp32; |out| ~ 0.15 so fp16's 0.05% quantization is
    # noise against the 2e-2 gate, and the output DMA traffic halves.
    out_d = nc.dram_tensor("out", [N, E], f16, kind="ExternalOutput").ap()
    # quarter-3's last outproj chunk, summed onto out rows 1536:2048 by
    # the host (which is already summing partial outputs across cores)
    outc3_d = nc.dram_tensor("outc3", [N // QH, E], f16, kind="ExternalOutput").ap()

    with tile.TileContext(nc) as tc:
        with (
            tc.tile_pool(name="persist", bufs=1) as persist,
            tc.tile_pool(name="pt_sb", bufs=8) as pt_sb,
            tc.tile_pool(name="small", bufs=3) as small,
            tc.tile_pool(name="ostage", bufs=4) as ostage,
        ):
            # ---- persistent SBUF tensors (DMA'd directly, all 16-bit) ----
            xT = persist.tile([P, ECH, N], f16, tag="xT")       # x^T
            qT = persist.tile([P, OCH, N], f16, tag="qT")       # (x Wq + bq)^T
            kT = persist.tile([P, OCH, N], f16, tag="kT")
            vaug = persist.tile([P, KC, HL, D + 1], bf16, tag="vaug")
            oT = persist.tile([P, OCH, N], f16, tag="oT")       # normalized O^T
            wq_s = persist.tile([P, ECH, EH], f16, tag="wq_s")
            wk_s = persist.tile([P, ECH, EH], f16, tag="wk_s")
            wv_s = persist.tile([P, ECH, EH], f16, tag="wv_s")
            wo_s = persist.tile([P, OCH, E], f16, tag="wo_s")
            bq_s = persist.tile([P, OCH], f32, tag="bq_s")
            bk_s = persist.tile([P, OCH], f32, tag="bk_s")
            # indicator for the pair-wide Z broadcast: pob = ind128^T @
            # zbuf replicates zbuf row 0 across partitions 0:64 and row 64
            # across 64:128 -- a K=128 matmul, so its LDWEIGHTS hides
            # under the surrounding K=128 AV/filler streams (K-class
            # changes force the PE to serialize the weight load).
            ind128 = persist.tile([P, P], bf16, tag="ind128")
            zbuf = persist.tile([P, QHW], bf16, tag="zbuf")
            warm = persist.tile([P, MV], bf16, tag="warm")
            thr = persist.tile([1, ECH, 1], f32, tag="thr")

            # memsets on the DVE: the gpsimd queue is busy issuing x DMA
            # descriptors for the first ~6us, the DVE is idle -- and the
            # warmup matmuls below gate on warm's memset
            nc.vector.memset(warm, 0.0)
            nc.vector.memset(ind128, 0.0)
            nc.vector.memset(zbuf, 0.0)
            nc.vector.memset(ind128[0:1, 0:D], 1.0)
            nc.vector.memset(ind128[D : D + 1, D : 2 * D], 1.0)
            # E/2-valued column of V_aug: the AV matmul's extra output row
            # is then (E/2)*sum(exp) = the softmax denominator pre-scaled
            nc.vector.memset(vaug[:, :, :, D : D + 1], float(E) / 2.0)

            # ---- input DMAs. Per-queue budgets balanced (~2.7MB each)
            # and ordered by first use: wv (V proj) and x quarter 0 land
            # first, wk next (K(0)), wq and the remaining x quarters
            # behind them. Whole-tensor dispatches (strided AP) keep the
            # per-queue descriptor count low -- each DMA_DIRECT2D costs
            # ~600ns of engine issue time. ----
            def dma_x(eng, q, parity):
                qsl = slice(q * QHW, (q + 1) * QHW)
                for c in range(parity, ECH, 2):
                    eng.dma_start(
                        out=xT[:, c, qsl], in_=xt_d[c * P : (c + 1) * P, qsl]
                    )

            def dma_w(eng, w_sb, w_dram, nch=ECH):
                # per-chunk dispatches: consumers start on the first
                # landed chunk instead of waiting for the whole tensor
                for c in range(nch):
                    eng.dma_start(
                        out=w_sb[:, c, :], in_=w_dram[c * P : (c + 1) * P, :]
                    )

            def dma_w1(eng, w_sb, w_dram, nch=ECH):
                eng.dma_start(
                    out=w_sb[:, 0:nch, :],
                    in_=w_dram.rearrange("(c p) e -> p c e", p=P),
                )

            # descriptors on one queue share its bandwidth round-robin, so
            # posting order is NOT priority. Instead, tiny "throttle" ops
            # that read one element of each just-loaded chunk stall the
            # issuing engine until that tensor lands -- later descriptors
            # aren't even posted until the earlier tensor has had the full
            # queue bandwidth. Dispatch cascade matches consumption order:
            # wv -> wk -> wq on scalar; xq0 -> xq1 -> xq2 -> xq3 on
            # gpsimd; the sparse sync queue carries xq0-e/xq3-e/wo flat.
            dma_x(nc.sync, 0, 0)
            dma_w(nc.scalar, wv_s, wv_d)
            nc.gpsimd.dma_start(out=bk_s, in_=bk_d)
            nc.gpsimd.dma_start(out=bq_s, in_=bq_d)
            dma_x(nc.gpsimd, 0, 1)

            nc.gpsimd.tensor_copy(
                out=thr[0:1, 0:4, 0:1], in_=xT[0:1, 1:8:2, 0:1]
            )
            dma_x(nc.gpsimd, 1, 0)
            dma_x(nc.gpsimd, 1, 1)
            nc.scalar.copy(out=thr[0:1, 0:8, 0:1], in_=wv_s[0:1, :, 0:1])
            dma_w(nc.scalar, wk_s, wk_d)

            nc.gpsimd.tensor_copy(
                out=thr[0:1, 0:8, 0:1], in_=xT[0:1, :, QHW : QHW + 1]
            )
            dma_x(nc.gpsimd, 2, 0)
            dma_x(nc.gpsimd, 2, 1)
            nc.scalar.copy(out=thr[0:1, 0:8, 0:1], in_=wk_s[0:1, :, 0:1])
            dma_w(nc.scalar, wq_s, wq_d)

            nc.gpsimd.tensor_copy(
                out=thr[0:1, 0:8, 0:1], in_=xT[0:1, :, 2 * QHW : 2 * QHW + 1]
            )
            dma_x(nc.gpsimd, 3, 0)
            dma_x(nc.gpsimd, 3, 1)
            nc.scalar.copy(out=thr[0:1, 0:8, 0:1], in_=wq_s[0:1, :, 0:1])
            dma_w1(nc.scalar, wo_s, wo_d, OCH)

            with (
                tc.tile_pool(name="psS", bufs=2, space="PSUM") as psS,
                tc.tile_pool(name="psO", bufs=2, space="PSUM") as psO,
                tc.tile_pool(name="psF", bufs=2, space="PSUM") as psF,
            ):
                # ---- HAM warmup: ~8 throwaway matmuls on a zeroed tile
                # start as soon as the engines boot (~2.5us), so the PE
                # clock-gate opens (1.2 -> 2.4 GHz after ~3.4us of
                # activity) before the first real projection lands ----
                # 10 MMs x ~427ns cold = 4.3us of activity: enough to trip
                # the 3.4us HAM busy-window (6 MMs = 2.6us was NOT -- the
                # gate stayed closed and the whole V-projection block ran
                # at 1.2GHz until ~23us)
                pw = psF.tile([P, MV], f32, tag="pf", name="pwarm")
                for r in range(10):
                    nc.tensor.matmul(
                        pw, lhsT=warm[:, 0:P], rhs=warm,
                        start=(r == 0), stop=(r == 9),
                    )
                # ---- filler machinery: projection/outproj work chopped
                # into single-PE-op closures, popped FILL per key-tile step
                # so the exp stream never starves behind a PE burst ----
                fillq = collections.deque()  # (req_pair_idx, unit_start, fn)

                def enqueue_unit(req, fns):
                    for k, fn in enumerate(fns):
                        fillq.append((req, k == 0, fn))

                def pop_fill(n=None, upto=None, finish_unit=False):
                    if finish_unit:
                        while fillq and not fillq[0][1]:
                            fillq.popleft()[2]()
                        return
                    if upto is not None:
                        while fillq and fillq[0][0] <= upto:
                            fillq.popleft()[2]()
                        return
                    for _ in range(n):
                        if not fillq:
                            return
                        fillq.popleft()[2]()

                def proj_unit_fns(w_sb, b_sb, dst, co, th):
                    sl = slice(th * MV, (th + 1) * MV)
                    cell = {}

                    def mk(ci):
                        def f():
                            if ci == 0:
                                cell["ps"] = psF.tile(
                                    [P, MV], f32, tag="pf", name="psproj"
                                )
                            nc.tensor.matmul(
                                cell["ps"],
                                lhsT=w_sb[:, ci, co * P : (co + 1) * P],
                                rhs=xT[:, ci, sl],
                                start=(ci == 0),
                                stop=(ci == ECH - 1),
                            )

                        return f

                    fns = [mk(ci) for ci in range(ECH)]

                    def cp():
                        nc.vector.tensor_scalar_add(
                            out=dst[:, co, sl],
                            in0=cell["ps"],
                            scalar1=b_sb[:, co : co + 1],
                        )

                    fns.append(cp)
                    return fns

                st_qs = [nc.sync, nc.scalar, nc.gpsimd]

                def outproj_unit_fns(t, eo, cmax=OCH):
                    """Output projection for token tile t, output half eo.
                    With cmax < OCH, accumulates chunks [0, cmax) and
                    stores the partial sum straight to out_d -- the tail
                    then adds chunk cmax.. via an accumulating DMA on the
                    SAME queue (per-queue FIFO orders the two stores)."""
                    DW = 512
                    esl = slice(eo * DW, (eo + 1) * DW)
                    tsl = slice(t * P, (t + 1) * P)
                    cell = {}

                    def mk(c):
                        def f():
                            if c == 0:
                                cell["ps"] = psF.tile(
                                    [P, DW], f32, tag="pf", name="psout"
                                )
                            nc.tensor.matmul(
                                cell["ps"],
                                lhsT=oT[:, c, tsl],
                                rhs=wo_s[:, c, esl],
                                start=(c == 0),
                                stop=(c == cmax - 1),
                            )

                        return f

                    fns = [mk(c) for c in range(cmax)]

                    def cp():
                        os_ = ostage.tile([P, DW], f16, tag="os")
                        nc.vector.tensor_copy(out=os_, in_=cell["ps"])
                        q = st_qs[t % 3] if cmax < OCH else nc.sync
                        q.dma_start(out=out_d[tsl, esl], in_=os_)

                    fns.append(cp)
                    return fns

                def enqueue_outproj(qq):
                    for t in range(qq * (KC // QH), (qq + 1) * (KC // QH)):
                        for eo in range(2):
                            enqueue_unit(10**6, outproj_unit_fns(t, eo))

                def enqueue_outproj_partial(qq):
                    for t in range(qq * (KC // QH), (qq + 1) * (KC // QH)):
                        for eo in range(2):
                            enqueue_unit(
                                10**6,
                                outproj_unit_fns(t, eo, cmax=OCH - 1),
                            )

                # ---- prefix: V projection and K(0) interleaved per token
                # quarter as its x^T lands, then Q(quarter 0, chunk 0) ----
                def vproj(t):
                    pv = psF.tile([P, EH], f32, tag="pf", name="pv")
                    for ci in range(ECH):
                        nc.tensor.matmul(
                            pv,
                            lhsT=xT[:, ci, t * P : (t + 1) * P],
                            rhs=wv_s[:, ci, :],
                            start=(ci == 0),
                            stop=(ci == ECH - 1),
                        )
                    nc.vector.tensor_copy(
                        out=vaug[:, t, :, 0:D],
                        in_=pv.rearrange("p (h d) -> p h d", h=HL),
                    )

                def vproj_unit_fns(t):
                    cell = {}

                    def mk(ci):
                        def f():
                            if ci == 0:
                                cell["ps"] = psF.tile(
                                    [P, EH], f32, tag="pf", name="pv"
                                )
                            nc.tensor.matmul(
                                cell["ps"],
                                lhsT=xT[:, ci, t * P : (t + 1) * P],
                                rhs=wv_s[:, ci, :],
                                start=(ci == 0),
                                stop=(ci == ECH - 1),
                            )

                        return f

                    fns = [mk(ci) for ci in range(ECH)]

                    def cp():
                        nc.vector.tensor_copy(
                            out=vaug[:, t, :, 0:D],
                            in_=cell["ps"].rearrange("p (h d) -> p h d", h=HL),
                        )

                    fns.append(cp)
                    return fns

                for q in range(QH):
                    for t in range(4 * q, min(4 * q + 4, 13)):
                        vproj(t)
                    if q < 3:
                        for fn in proj_unit_fns(wk_s, bk_s, kT, 0, q):
                            fn()
                    if q == 1:
                        for fn in proj_unit_fns(wq_s, bq_s, qT, 0, 0):
                            fn()
                # the rest of pair (0,0)'s needs drain as the first fillers:
                # K(0) tokens 1536-2048 (used from step 10), then V13-15
                # (used at steps 13-15) -- FILL=3/step clears them in time
                enqueue_unit(0, proj_unit_fns(wk_s, bk_s, kT, 0, 3))
                for t in range(13, KC):
                    enqueue_unit(0, vproj_unit_fns(t))

                # remaining projections become fillers, FIFO in deadline
                # order: K(j)/Q(0,j) before pair (0,j), Q(qq,j) before
                # pair 4*qq+j
                for j in range(1, HL // 2):
                    for th in range(N // MV):
                        enqueue_unit(j, proj_unit_fns(wk_s, bk_s, kT, j, th))
                    enqueue_unit(j, proj_unit_fns(wq_s, bq_s, qT, j, 0))
                for qq in range(1, QH):
                    for j in range(HL // 2):
                        enqueue_unit(
                            4 * qq + j, proj_unit_fns(wq_s, bq_s, qT, j, qq)
                        )

                def s_pair_for(j, qq, kc):
                    qsl = slice(qq * QHW, (qq + 1) * QHW)
                    ss = psS.tile([P, 2 * QHW], f32, tag="ss")
                    ksl = slice(kc * P, (kc + 1) * P)
                    nc.tensor.matmul(
                        ss[:, 0:QHW],
                        lhsT=kT[0:D, j, ksl],
                        rhs=qT[0:D, j, qsl],
                        start=True,
                        stop=True,
                    )
                    nc.tensor.matmul(
                        ss[:, QHW : 2 * QHW],
                        lhsT=kT[D : 2 * D, j, ksl],
                        rhs=qT[D : 2 * D, j, qsl],
                        start=True,
                        stop=True,
                    )
                    return ss

                def drain_pair(j, ocp, pqq):
                    """Normalize one pair's accumulated O^T into oT. Both
                    heads' pre-scaled Z rows (parked in zbuf partitions 0
                    and 64) are broadcast across their 64-partition halves
                    by a single K=128 indicator matmul, inverted with the
                    fast-approx reciprocal, and one DVE multiply writes
                    both heads of oT. Emitted deep inside the NEXT pair's
                    loop, between K=128 AV/filler matmuls so the LDW
                    hides."""
                    qsl = slice(pqq * QHW, (pqq + 1) * QHW)
                    zinv = small.tile([P, QHW], f32, tag="zinv")
                    pob = psF.tile([P, QHW], f32, tag="pf", name="pob")
                    nc.tensor.matmul(
                        pob, lhsT=ind128, rhs=zbuf, start=True, stop=True
                    )
                    nc.vector.reciprocal_approx_fast(out=zinv, in_=pob)
                    nc.vector.tensor_tensor(
                        out=oT[:, j, qsl], in0=ocp, in1=zinv, op=mult
                    )

                def attn_pair(
                    j, qq, pending, preS, nxt, nxt_idx, after_drain,
                    steps_left, fill=FILL, last=False,
                ):
                    def fill_rate(kc):
                        return fill
                    """S^T/exp/O for heads (2j, 2j+1) on quarter qq. S-pairs
                    run two steps ahead of the O-pairs (and preload into the
                    NEXT pair at kc 14/15) so ScalarE's exp stream never
                    waits on the PE's static order; the previous pair's
                    normalization drains mid-loop; filler closures soak up
                    the per-step PE slack."""
                    po_e = psO.tile([P, QHW], f32, tag="po")
                    po_o = psO.tile([P, QHW], f32, tag="po")
                    sss = (
                        preS
                        if preS is not None
                        else [s_pair_for(j, qq, 0), s_pair_for(j, qq, 1)]
                    )
                    nxtS = []
                    pts = []

                    def av_pair(kc):
                        nc.tensor.matmul(
                            po_e[0 : D + 1, :],
                            lhsT=vaug[:, kc, 2 * j, :],
                            rhs=pts[kc][:, 0:QHW],
                            start=(kc == 0),
                            stop=(kc == KC - 1),
                        )
                        nc.tensor.matmul(
                            po_o[0 : D + 1, :],
                            lhsT=vaug[:, kc, 2 * j + 1, :],
                            rhs=pts[kc][:, QHW : 2 * QHW],
                            start=(kc == 0),
                            stop=(kc == KC - 1),
                        )

                    for kc in range(KC):
                        pT = pt_sb.tile([P, 2 * QHW], bf16, tag="pT")
                        nc.scalar.activation(pT, sss[kc], Exp)
                        pts.append(pT)
                        if kc + 2 < KC:
                            sss.append(s_pair_for(j, qq, kc + 2))
                        # AV lags exp by one step: its pT finished a full
                        # step ago (no ScalarE sem wait), and the first AV
                        # of a pair lands after the previous pair's PSUM-
                        # freeing copies have drained (no psO WAR stall)
                        if kc >= 1:
                            av_pair(kc - 1)
                        if kc == 5 and pending:
                            # psF slot discipline: finish any half-
                            # emitted filler unit before pob allocs
                            pop_fill(finish_unit=True)
                            for args in pending:
                                drain_pair(*args)
                            pending.clear()
                            if after_drain is not None:
                                after_drain()
                        if nxt is not None and kc >= KC - 2:
                            if kc == KC - 2:
                                # anything the next pair depends on must be
                                # emitted before its S tiles start
                                pop_fill(upto=nxt_idx)
                            nq, njj = nxt
                            nxtS.append(s_pair_for(njj, nq, kc - (KC - 2)))
                        pop_fill(fill_rate(kc))
                    av_pair(KC - 1)
                    if last:
                        # ScalarE is done with exps: keep the PE's clock
                        # gate open through the drain with throwaway
                        # matmuls, and split the PSUM-freeing copies
                        # across ScalarE/DVE so the tail chain starts ~2x
                        # sooner
                        for r in range(8):
                            pwt = psF.tile([P, MV], f32, tag="pf", name="pw2")
                            nc.tensor.matmul(
                                pwt, lhsT=warm[:, 0:P], rhs=warm,
                                start=True, stop=True,
                            )
                        nc.vector.tensor_copy(
                            out=zbuf[0:1, :], in_=po_e[D : D + 1, :]
                        )
                        nc.scalar.copy(
                            out=zbuf[D : D + 1, :], in_=po_o[D : D + 1, :]
                        )
                        ocp = small.tile([P, QHW], bf16, tag="ocp")
                        nc.vector.tensor_copy(
                            out=ocp[0:D, :], in_=po_e[0:D, :]
                        )
                        nc.scalar.copy(out=ocp[D:P, :], in_=po_o[0:D, :])
                        return [(j, ocp, qq)], nxtS
                    # quick copies free the PSUM accumulators: both Z rows
                    # into zbuf (partitions 0/64, read by the indicator
                    # broadcast at the next pair's drain), both O^T halves
                    # into one [128, QHW] tile
                    nc.vector.tensor_copy(
                        out=zbuf[0:1, :], in_=po_e[D : D + 1, :]
                    )
                    nc.vector.tensor_copy(
                        out=zbuf[D : D + 1, :], in_=po_o[D : D + 1, :]
                    )
                    ocp = small.tile([P, QHW], bf16, tag="ocp")
                    nc.vector.tensor_copy(out=ocp[0:D, :], in_=po_e[0:D, :])
                    nc.vector.tensor_copy(out=ocp[D:P, :], in_=po_o[0:D, :])
                    return [(j, ocp, qq)], nxtS

                pairs = [(qq, j) for qq in range(QH) for j in range(HL // 2)]
                pending, preS = [], None
                for idx, (qq, j) in enumerate(pairs):
                    nxt = pairs[idx + 1] if idx + 1 < len(pairs) else None
                    after_drain = None
                    if j == 0 and qq >= 1:
                        after_drain = (lambda q=qq - 1: enqueue_outproj(q))
                    elif (qq, j) == (QH - 1, HL // 2 - 1):
                        # last pair: pre-accumulate quarter 3's outproj over
                        # head chunks 0-2 while this pair runs
                        after_drain = (lambda: enqueue_outproj_partial(QH - 1))
                    last = idx == len(pairs) - 1
                    # pair 0: high fill so the K(1)/Q(0,1) backlog doesn't
                    # flush serially (9us of exp-idle) at pair (0,1)'s
                    # start; last pair: drain the quarter-3 outproj
                    # partials while it runs so the tail starts at once;
                    # otherwise FILL=2 ~= the average filler arrival rate,
                    # which keeps the queue nonempty across whole pairs
                    # (bursty flushes leave later exp-bound steps with no
                    # filler work to hide).
                    pending, preS = attn_pair(
                        j, qq, pending, preS, nxt, idx + 1, after_drain,
                        (len(pairs) - idx) * KC,
                        fill=(
                            5
                            if (last or idx == 0)
                            else FILL
                        ),
                        last=last,
                    )
                # drain FIRST (after closing any half-emitted psF unit);
                # the leftover filler backlog flushes after the tail's
                # chunk-3 matmuls below, overlapping the tail's copies
                # and stores instead of delaying the whole chain
                pop_fill(finish_unit=True)
                for args in pending:
                    drain_pair(*args)
                pending.clear()
                # tail: the last head chunk's outproj for quarter 3. Both
                # e-halves of a token tile go into one [P, 1024] PSUM pod
                # (psS pool -- free once the last exp is done, same tile
                # shape as the S tiles), copies alternate ScalarE/DVE, and
                # the result lands in its own output (outc3) that the host
                # adds onto out rows 1536:2048 -- so the partial stores
                # need no ordering against these.
                _ = add  # (kept: AluOpType import used by filler adds)
                # bridge dummies: the copy/drain gaps just before this
                # loop re-throttled the clock gate, making these matmuls
                # run at 1.2GHz -- keep the PE nominally busy instead
                for r in range(3):
                    pwt = psF.tile([P, MV], f32, tag="pf", name="pw3")
                    nc.tensor.matmul(
                        pwt, lhsT=warm[:, 0:P], rhs=warm,
                        start=True, stop=True,
                    )
                for ti, t in enumerate(
                    range((QH - 1) * (KC // QH), QH * (KC // QH))
                ):
                    tsl = slice(t * P, (t + 1) * P)
                    pod = psS.tile([P, 2 * QHW], f32, tag="ss", name="podl")
                    for eo in range(2):
                        nc.tensor.matmul(
                            pod[:, eo * 512 : (eo + 1) * 512],
                            lhsT=oT[:, OCH - 1, tsl],
                            rhs=wo_s[:, OCH - 1, eo * 512 : (eo + 1) * 512],
                            start=True,
                            stop=True,
                        )
                    os_ = ostage.tile([P, 2 * QHW], f16, tag="os2", bufs=2)
                    # last tile's copy on ScalarE (1147ns vs DVE ~1370)
                    # so the final serial chain is the shorter one
                    if ti % 2 == 1:
                        nc.scalar.copy(out=os_, in_=pod)
                    else:
                        nc.vector.tensor_copy(out=os_, in_=pod)
                    # queue balance: the quarter-3 partial stores used
                    # sync twice (t=12,15); keep outc3 off the doubly-
                    # loaded queue so no queue carries 3 of the final
                    # transfers
                    [nc.scalar, nc.gpsimd, nc.scalar, nc.gpsimd][
                        ti
                    ].dma_start(
                        out=outc3_d[ti * P : (ti + 1) * P, :], in_=os_
                    )
                # flush the remaining filler backlog (quarter-3 partial
                # stores etc.) on the PE while the tail's copies and
                # outc3 stores complete on ScalarE/DVE/the DMA queues
                pop_fill(upto=10**6)
    nc.compile()
    return nc


def _get_nc():
    if "nc" not in _CACHE:
        _CACHE["nc"] = _build()
    return _CACHE["nc"]


def _make_in_maps(x, Wq, bq, Wk, bk, Wv, bv, Wo, bo):
    x = np.asarray(x, dtype=np.float32)
    Wq = np.asarray(Wq, dtype=np.float16)
    Wk = np.asarray(Wk, dtype=np.float16)
    Wv = np.asarray(Wv, dtype=np.float16)
    Wo = np.asarray(Wo, dtype=np.float16)
    bq = np.asarray(bq, dtype=np.float32)
    bk = np.asarray(bk, dtype=np.float32)

    xts = [np.ascontiguousarray(x[b].T.astype(np.float16)) for b in range(B)]
    in_maps = []
    for c in range(8):
        b, hh = divmod(c, 2)
        sl = slice(hh * EH, (hh + 1) * EH)
        in_maps.append(
            {
                "xt": xts[b],
                "wq": np.ascontiguousarray(Wq[:, sl]),
                "wk": np.ascontiguousarray(Wk[:, sl]),
                "wv": np.ascontiguousarray(Wv[:, sl]),
                "wo": np.ascontiguousarray(Wo[sl, :]),
                "bqc": np.ascontiguousarray(bq[sl].reshape(OCH, P).T),
                "bkc": np.ascontiguousarray(bk[sl].reshape(OCH, P).T),
            }
        )
    return in_maps


def kernel(x, Wq, bq, Wk, bk, Wv, bv, Wo, bo):
    from concourse.bass_utils import run_bass_kernel_spmd

    Wo32 = np.asarray(Wo, dtype=np.float32)
    bv32 = np.asarray(bv, dtype=np.float32)
    bo32 = np.asarray(bo, dtype=np.float32)

    nc = _get_nc()
    in_maps = _make_in_maps(x, Wq, bq, Wk, bk, Wv, bv, Wo, bo)
    res = run_bass_kernel_spmd(nc, in_maps, list(range(8))).results

    # Exact bias correction: softmax rows sum to 1, so A rows sum to 1/512
    # and the V-bias term is the constant row (bv/512) @ Wo; bo likewise.
    corr = (
        bv32.astype(np.float64) @ Wo32.astype(np.float64) / (E / 2.0)
        + bo32.astype(np.float64)
    ).astype(np.float32)

    out = np.empty((B, N, E), dtype=np.float32)
    q3 = slice(N - N // QH, N)
    for b in range(B):
        out[b] = (
            res[2 * b]["out"].astype(np.float32)
            + res[2 * b + 1]["out"].astype(np.float32)
            + corr[None, :]
        )
        out[b, q3] += res[2 * b]["outc3"].astype(np.float32) + res[
            2 * b + 1
        ]["outc3"].astype(np.float32)
    return out

